# revision 21
# baseline (speedup 1.0000x reference)
"""Trainium2 Bass kernel for GQA attention (B=4, S=2048, D=768, H=12, KVH=4, HD=64).

Sharding: 2 cores per batch. Each core computes all 12 heads for 4 query
chunks of 256 rows (role 0: chunks {0,2,4,6}, role 1: {1,3,5,7}) against the
full K/V of its batch, plus the complete wo projection for its rows. Output
shards are concatenated on the host; no collectives.

All cores run the same graph; causal structure differences between roles are
data-driven (host-built multiplicative masks applied to exp(scores)).

On-chip layout is transposed: qT/kT [head_dim, seq] (scores come out k-major),
V natural [seq, head_dim] with a 65th all-ones column per kv head so the PV
matmul also produces the softmax denominator in psum row 64 (no separate
denominator matmuls). Normalization: reciprocal on the psum denominator row,
DMA partition-broadcast of the reciprocal, elementwise multiply; odd heads of
each pair are shifted to partitions 64-127 of attnT by a small SBUF-to-SBUF
DMA. RoPE pairs are deinterleaved (host-permuted wq/wk columns) so the complex
rotation becomes out = cos*t + sign * SWAP @ (sin*t) on 1024-wide chunks.
"""

import sys

if "/opt/trn_rl_repo" not in sys.path:
    sys.path.insert(0, "/opt/trn_rl_repo")

import numpy as np
import ml_dtypes

import concourse.bass as bass
import concourse.tile as tile
from concourse import bacc, mybir

F32 = mybir.dt.float32
BF16 = mybir.dt.bfloat16

B, S, D = 4, 2048, 768
H, KVH, HD = 12, 4, 64

PAIRS = [(0, 1), (2, 3), (4, 5), (6, 7), (8, 9), (10, 11)]
GROUPS = [(0, 1), (2, 3), (4, 5)]  # pair indices per exp-batch group


def _kv(h):
    return h // 3


def _dram_view(base_ap, ap_list, extra_offset=0):
    return bass.AP(tensor=base_ap.tensor, offset=base_ap.offset + extra_offset,
                   ap=ap_list)


def build_nc(phases=3):
    nc = bacc.Bacc(None, target_bir_lowering=False)

    xT = nc.dram_tensor("xT", [D, S], BF16, kind="ExternalInput")
    xqT = nc.dram_tensor("xqT", [D, 1024], BF16, kind="ExternalInput")
    wq = nc.dram_tensor("wq", [D, H * HD], BF16, kind="ExternalInput")
    wk = nc.dram_tensor("wk", [D, KVH * HD], BF16, kind="ExternalInput")
    wv = nc.dram_tensor("wv", [D, KVH * HD], BF16, kind="ExternalInput")
    wo = nc.dram_tensor("wo", [H * HD, D], BF16, kind="ExternalInput")
    kcs = nc.dram_tensor("kcs", [64, S], BF16, kind="ExternalInput")  # [cos;sin]
    qcs = nc.dram_tensor("qcs", [64, 1024], BF16, kind="ExternalInput")
    masks = nc.dram_tensor("masks", [16, 128, 256], BF16, kind="ExternalInput")
    out = nc.dram_tensor("out", [D, 1024], BF16, kind="ExternalOutput")

    # swap matrix: exchanges 32-partition blocks 0<->1, 2<->3
    SW = np.zeros((128, 128), ml_dtypes.bfloat16)
    for blk in range(4):
        srcb = blk ^ 1
        for i in range(32):
            SW[blk * 32 + i, srcb * 32 + i] = 1.0
    sw_dram = nc.inline_tensor(SW, name="swconst")
    sign = np.zeros((128, 1), np.float32)
    for blk in range(4):
        sign[blk * 32:(blk + 1) * 32] = -1.0 if blk % 2 == 0 else 1.0
    sign_dram = nc.inline_tensor(sign, name="signconst")
    id_dram = nc.inline_tensor(np.eye(128, dtype=ml_dtypes.bfloat16),
                               name="idconst")

    def mm(out_ap, lhsT, rhs, start, stop, tile_position=None):
        nc.tensor.matmul(
            out_ap, lhsT, rhs,
            start=start, stop=stop,
            tile_position=tile_position,
            skip_group_check=True,
        )

    with tile.TileContext(nc) as tc:
        with tc.tile_pool(name="persist", bufs=1) as persist:
            qT64 = persist.tile([64, H, 1024], BF16)
            kT64 = persist.tile([64, KVH, S], BF16)
            # V with a 65th all-ones column per kv head (denominator trick)
            V2 = persist.tile([128, 16, KVH, 65], BF16)
            wk_sb = persist.tile([128, 6, 256], BF16)
            wv_sb = persist.tile([128, 6, 256], BF16)
            wo_sb = persist.tile([128, 6, D], BF16)
            sw_sb = persist.tile([128, 128], BF16)
            id_sb = persist.tile([128, 128], BF16)
            sign_sb = persist.tile([128, 1], F32)

            def load_rows(dst, src_dram, ncols, nblk, col0=0, width=None):
                # [nblk*128, ncols] dram -> [128, nblk, width] sbuf, one DMA
                width = ncols if width is None else width
                src = _dram_view(src_dram[:, :],
                                 [[ncols, 128], [128 * ncols, nblk],
                                  [1, width]], extra_offset=col0)
                nc.sync.dma_start(out=dst, in_=src)

            nc.vector.memset(V2[:, :, :, :], 1.0)  # ones col survives at 64
            load_rows(wk_sb[:, :, :], wk, 256, 6)
            load_rows(wv_sb[:, :, :], wv, 256, 6)
            nc.sync.dma_start(out=sw_sb[:, :], in_=sw_dram[:, :])
            nc.sync.dma_start(out=id_sb[:, :], in_=id_dram[:, :])
            nc.sync.dma_start(out=sign_sb[:, :], in_=sign_dram[:, :])

            # ---------------- Phase 1: projections + rope ----------------
            with tc.tile_pool(name="p1", bufs=1) as p1, \
                 tc.tile_pool(name="cs", bufs=2) as csp, \
                 tc.tile_pool(name="tmp", bufs=3) as tmpp, \
                 tc.tile_pool(name="rop", bufs=3) as ropp, \
                 tc.tile_pool(name="psA", bufs=2, space="PSUM") as psA, \
                 tc.tile_pool(name="psB", bufs=2, space="PSUM") as psB:

                xT_sb = p1.tile([128, 6, S], BF16)
                xqT_sb = p1.tile([128, 6, 1024], BF16)
                wq_sb = p1.tile([128, 6, H * HD], BF16)
                cs_q = p1.tile([128, 2, 1024], BF16)

                # xT in 4 column chunks so compute starts early
                for c in range(4):
                    load_rows(xT_sb[:, :, c * 512:(c + 1) * 512], xT, S, 6,
                              col0=c * 512, width=512)
                load_rows(xqT_sb[:, :, :], xqT, 1024, 6)
                load_rows(wq_sb[:, :, :], wq, H * HD, 6)
                load_rows(wo_sb[:, :, :], wo, D, 6)

                def cs_load(dst, src_dram, col0, width):
                    # [cos(32 rows); sin(32 rows)] x width -> [128, 2, width]
                    # partitions replicate each 32-row block 4x
                    ncols = src_dram.shape[1]
                    for cs in range(2):
                        src = _dram_view(
                            src_dram[:, :],
                            [[0, 4], [ncols, 32], [1, width]],
                            extra_offset=col0 + cs * 32 * ncols)
                        nc.sync.dma_start(out=dst[:, cs, :], in_=src)

                warm_ps = psA.tile([128, 1024], F32, tag="pj")
                for wi in range(24):
                    mm(warm_ps[:, 0:128], id_sb[:, :], id_sb[:, :],
                       start=(wi == 0), stop=(wi == 23))
                warm_sb = tmpp.tile([128, 1024], F32, tag="ta")
                nc.scalar.activation(
                    out=warm_sb[:, 0:128], in_=warm_ps[:, 0:128],
                    func=mybir.ActivationFunctionType.Exp, scale=0.01)

                def rope_chunk(proj_ps, cs_t):
                    # returns bf16 [128, 1024]: cos*t + sign * SWAP @ (sin*t)
                    ta = tmpp.tile([128, 1024], F32, tag="ta")
                    tb = tmpp.tile([128, 1024], BF16, tag="tb")
                    nc.vector.tensor_mul(ta[:, :], proj_ps, cs_t[:, 0, :])
                    nc.vector.tensor_mul(tb[:, :], proj_ps, cs_t[:, 1, :])
                    sw_ps = psB.tile([128, 1024], F32, tag="sw")
                    mm(sw_ps[:, 0:512], sw_sb[:, :], tb[:, 0:512],
                       start=True, stop=True)
                    mm(sw_ps[:, 512:1024], sw_sb[:, :], tb[:, 512:1024],
                       start=True, stop=True)
                    ro = ropp.tile([128, 1024], BF16, tag="ro")
                    nc.vector.scalar_tensor_tensor(
                        out=ro[:, :],
                        in0=sw_ps[:, :],
                        scalar=sign_sb[:, 0:1],
                        in1=ta[:, :],
                        op0=mybir.AluOpType.mult,
                        op1=mybir.AluOpType.add,
                    )
                    return ro

                # K projection + rope: m-tile = kv pair (2m, 2m+1); 2 chunks
                for c in range(2):
                    cs_t = csp.tile([128, 2, 1024], BF16, tag="cs")
                    cs_load(cs_t[:, :, :], kcs, c * 1024, 1024)
                    for m in range(2):
                        k_ps = psA.tile([128, 1024], F32, tag="pj")
                        for dt in range(6):
                            for hf in range(2):
                                mm(k_ps[:, hf * 512:(hf + 1) * 512],
                                   wk_sb[:, dt, m * 128:(m + 1) * 128],
                                   xT_sb[:, dt, c * 1024 + hf * 512:
                                         c * 1024 + (hf + 1) * 512],
                                   start=(dt == 0), stop=(dt == 5))
                        ro = rope_chunk(k_ps[:, :], cs_t)
                        nc.sync.dma_start(
                            out=kT64[:, 2 * m, c * 1024:(c + 1) * 1024],
                            in_=ro[0:64, :])
                        nc.sync.dma_start(
                            out=kT64[:, 2 * m + 1, c * 1024:(c + 1) * 1024],
                            in_=ro[64:128, :])

                # Q projection + rope: m-tile = head pair (2t, 2t+1)
                cs_load(cs_q[:, :, :], qcs, 0, 1024)
                for t in range(6):
                    q_ps = psA.tile([128, 1024], F32, tag="pj")
                    for dt in range(6):
                        for hf in range(2):
                            mm(q_ps[:, hf * 512:(hf + 1) * 512],
                               wq_sb[:, dt, t * 128:(t + 1) * 128],
                               xqT_sb[:, dt, hf * 512:(hf + 1) * 512],
                               start=(dt == 0), stop=(dt == 5))
                    ro = rope_chunk(q_ps[:, :], cs_q)
                    nc.sync.dma_start(out=qT64[:, 2 * t, :], in_=ro[0:64, :])
                    nc.sync.dma_start(out=qT64[:, 2 * t + 1, :],
                                      in_=ro[64:128, :])

                # V projection (natural layout): 16 seq tiles, 4 per psum
                for v4 in range(4):
                    v_ps = psA.tile([128, 4, 4, 64], F32, tag="pj")
                    for st4 in range(4):
                        st = v4 * 4 + st4
                        for dt in range(6):
                            mm(v_ps[:, st4, :, :],
                               xT_sb[:, dt, st * 128:(st + 1) * 128],
                               wv_sb[:, dt, :], start=(dt == 0),
                               stop=(dt == 5))
                    if phases != 7:  # 7: debug, leave V2 = memset ones
                        for st4 in range(4):
                            nc.scalar.copy(V2[:, v4 * 4 + st4, :, 0:64],
                                           v_ps[:, st4, :, :])

            if phases == 1:
                for t in range(6):
                    nc.sync.dma_start(
                        out=out[t * 128:t * 128 + 64, :],
                        in_=qT64[:, 2 * t, :])
                    nc.sync.dma_start(
                        out=out[t * 128 + 64:(t + 1) * 128, :],
                        in_=qT64[:, 2 * t + 1, :])

            if phases in (6, 7):  # debug: dump V2 (first 6 seq tiles)
                for st in range(6):
                    nc.sync.dma_start(
                        out=out[st * 128:(st + 1) * 128, 0:260],
                        in_=V2[:, st, :, :])

            # ---------------- Phase 2: attention ----------------
            if 2 <= phases <= 5:
              with tc.tile_pool(name="p2", bufs=1) as p2, \
                 tc.tile_pool(name="expp", bufs=3) as expp, \
                 tc.tile_pool(name="nrm", bufs=3) as nrm, \
                 tc.tile_pool(name="zsb", bufs=2) as zsb, \
                 tc.tile_pool(name="recdp", bufs=3, space="DRAM") as recdp, \
                 tc.tile_pool(name="psSC", bufs=2, space="PSUM") as psSC, \
                 tc.tile_pool(name="psPV", bufs=4, space="PSUM") as psPV:

                masks_sb = p2.tile([128, 16, 256], BF16)
                attnT = p2.tile([128, 6, 1024], BF16)
                src = _dram_view(masks[:, :, :],
                                 [[256, 128], [128 * 256, 16], [1, 256]])
                nc.sync.dma_start(out=masks_sb[:, :, :], in_=src)

                def rep4(base):
                    # [128, 256] -> [128, 4, 256] via stride-0 free dim
                    return bass.AP(tensor=base.tensor, offset=base.offset,
                                   ap=[list(base.ap)[0], [0, 4], [1, 256]])

                def quad(base):
                    # [128, 1024] contiguous -> [128, 4, 256] view
                    return bass.AP(tensor=base.tensor, offset=base.offset,
                                   ap=[list(base.ap)[0], [256, 4], [1, 256]])

                # deferred wo emission: one mi-tile per call
                def wo_mi(c, mi, z_all):
                    z_ps = psSC.tile([128, 256], F32, tag="sc")
                    for hd in range(6):
                        mm(z_ps[:, :],
                           wo_sb[:, hd, mi * 128:(mi + 1) * 128],
                           attnT[:, hd, c * 256:(c + 1) * 256],
                           start=(hd == 0), stop=(hd == 5))
                    nc.vector.tensor_copy(z_all[:, mi, :], z_ps[:, :])

                def wo_flush(c, z_all):
                    dst = _dram_view(out[:, :],
                                     [[1024, 128], [128 * 1024, 6], [1, 256]],
                                     extra_offset=c * 256)
                    nc.sync.dma_start(out=dst, in_=z_all[:, :, :])

                pending_wo = []  # ("mi"|"flush", fn) emission queue

                def pop_wo(n):
                    done = 0
                    while pending_wo and done < n:
                        kind, fn = pending_wo[0]
                        if kind == "mi":
                            done += 1
                        pending_wo.pop(0)
                        fn()

                for s in range(4):
                    n_kt = 4 * s + 4
                    for gi, grp in enumerate(GROUPS):
                        pvt = [psPV.tile([128, 512], F32, tag="pv",
                                         name=f"pv{s}{gi}{pi}")
                               for pi in range(2)]
                        for kt in range(n_kt):
                            if kt % 2 == 0:
                                expT2 = expp.tile([128, 2, 1024], BF16,
                                                  tag="expT")
                            sc_ps = psSC.tile([128, 1024], F32, tag="sc")
                            masked = kt >= 4 * s
                            # scores (transposed): [k 128, q 256] per head
                            for pi, p in enumerate(grp):
                                hA, hB = PAIRS[p]
                                o = pi * 512
                                if _kv(hA) == _kv(hB):
                                    mm(sc_ps[:, o:o + 512],
                                       kT64[:, _kv(hA),
                                            kt * 128:(kt + 1) * 128],
                                       qT64[:, hA:hA + 2,
                                            s * 256:(s + 1) * 256],
                                       start=True, stop=True)
                                else:
                                    for half, h in enumerate((hA, hB)):
                                        mm(sc_ps[:, o + half * 256:
                                                 o + half * 256 + 256],
                                           kT64[:, _kv(h),
                                                kt * 128:(kt + 1) * 128],
                                           qT64[:, h,
                                                s * 256:(s + 1) * 256],
                                           start=(half == 0), stop=True)
                            # exp (scale 1/8) from psum -> sbuf
                            expT = expT2[:, kt % 2, :]
                            nc.scalar.activation(
                                out=expT,
                                in_=sc_ps[:, :],
                                func=mybir.ActivationFunctionType.Exp,
                                scale=0.125,
                            )
                            if masked:
                                midx = s * 4 + (kt - 4 * s)
                                nc.vector.tensor_mul(
                                    quad(expT), quad(expT),
                                    rep4(masks_sb[:, midx, :]))
                            # PV with ones column: row 64 = denominator
                            for pi, p in enumerate(grp):
                                hA, hB = PAIRS[p]
                                kvA, kvB = _kv(hA), _kv(hB)
                                o = pi * 512
                                if kvA == kvB:
                                    mm(pvt[pi][0:65, 0:512],
                                       V2[:, kt, kvA, :],
                                       expT2[:, kt % 2, o:o + 512],
                                       start=(kt == 0),
                                       stop=(kt == n_kt - 1))
                                else:
                                    mm(pvt[pi][0:65, 0:256],
                                       V2[:, kt, kvA, :],
                                       expT2[:, kt % 2, o:o + 256],
                                       start=(kt == 0),
                                       stop=(kt == n_kt - 1))
                                    mm(pvt[pi][0:65, 256:512],
                                       V2[:, kt, kvB, :],
                                       expT2[:, kt % 2, o + 256:o + 512],
                                       start=False,
                                       stop=(kt == n_kt - 1))
                        # ---- normalize group ----
                        denb = nrm.tile([64, 2, 512], F32, tag="denb",
                                        name=f"denb{s}{gi}")
                        recb = nrm.tile([64, 2, 512], F32, tag="recb",
                                        name=f"recb{s}{gi}")
                        tmpB = nrm.tile([64, 2, 256], BF16, tag="tmpB",
                                        name=f"tmpB{s}{gi}")
                        # denominator rows (psum lane 64) -> DRAM bounce ->
                        # broadcast to 64 partitions, then reciprocal at base 0
                        # (reciprocal_approx_fast misbehaves off partition 0)
                        recd_t = recdp.tile([2, 512], F32, tag="recd",
                                            name=f"recd{s}{gi}")
                        den_sb = nrm.tile([128, 2, 512], F32, tag="den",
                                          name=f"den{s}{gi}")
                        for pi in range(2):
                            nc.vector.tensor_copy(den_sb[64:65, pi, :],
                                                  pvt[pi][64:65, 0:512])
                        nc.sync.dma_start(out=recd_t[:, :],
                                          in_=den_sb[64:65, :, :])
                        base = recd_t[:, :]
                        rep = bass.AP(tensor=base.tensor, offset=base.offset,
                                      ap=[[0, 64]] + list(base.ap))
                        nc.sync.dma_start(out=denb[:, :, :], in_=rep)
                        nc.vector.reciprocal_approx_fast(
                            recb[:, :, :], denb[:, :, :])
                        qs = slice(s * 256, (s + 1) * 256)
                        for pi, p in enumerate(grp):
                            nc.vector.tensor_mul(
                                attnT[0:64, p, qs],
                                pvt[pi][0:64, 0:256], recb[:, pi, 0:256])
                            nc.vector.tensor_mul(
                                tmpB[:, pi, :],
                                pvt[pi][0:64, 256:512], recb[:, pi, 256:512])
                        # shift odd heads to partitions 64-127 of attnT
                        nc.sync.dma_start(
                            out=attnT[64:128, grp[0]:grp[0] + 2, qs],
                            in_=tmpB[:, :, :])
                        # 2 deferred wo matmul groups per group boundary
                        pop_wo(2)
                    # queue wo chunk for this s (emitted at group boundaries)
                    if phases >= 3:
                        z_all = zsb.tile([128, 6, 256], BF16, tag="z",
                                         name=f"zall{s}")
                        for mi in range(6):
                            pending_wo.append(
                                ("mi", lambda c=s, mi=mi, z=z_all:
                                 wo_mi(c, mi, z)))
                        pending_wo.append(
                            ("flush", lambda c=s, z=z_all: wo_flush(c, z)))
                # drain remaining wo work (last s)
                for _, fn in pending_wo:
                    fn()
                pending_wo = []

                if phases == 2:
                    for t in range(6):
                        nc.sync.dma_start(
                            out=out[t * 128:(t + 1) * 128, :],
                            in_=attnT[:, t, :])

    nc.compile()
    return nc


# ---------------------------------------------------------------------------
# host side
# ---------------------------------------------------------------------------

def _permute_cols(w, nheads):
    """Deinterleave rope pairs within each head: even dims then odd dims."""
    cols = []
    for h in range(nheads):
        blk = w[:, h * HD:(h + 1) * HD]
        cols.append(blk[:, 0::2])
        cols.append(blk[:, 1::2])
    return np.ascontiguousarray(np.concatenate(cols, axis=1))


def make_in_maps(x, wq, wk, wv, wo, freqs_cos, freqs_sin):
    bf = ml_dtypes.bfloat16
    wq_p = _permute_cols(np.asarray(wq, np.float32), H).astype(bf)
    wk_p = _permute_cols(np.asarray(wk, np.float32), KVH).astype(bf)
    wv_b = np.ascontiguousarray(np.asarray(wv, np.float32)).astype(bf)
    wo_b = np.ascontiguousarray(np.asarray(wo, np.float32)).astype(bf)

    cosT = np.ascontiguousarray(np.asarray(freqs_cos, np.float32).T)  # [32, S]
    sinT = np.ascontiguousarray(np.asarray(freqs_sin, np.float32).T)
    kcs = np.ascontiguousarray(np.concatenate([cosT, sinT], axis=0))  # [64, S]

    in_maps = []
    for core in range(8):
        b, role = core // 2, core % 2
        xT = np.ascontiguousarray(np.asarray(x[b], np.float32).T)
        q_rows = np.concatenate(
            [np.arange(256 * (2 * s + role), 256 * (2 * s + role) + 256)
             for s in range(4)])
        xqT = np.ascontiguousarray(xT[:, q_rows])
        qcs = np.ascontiguousarray(kcs[:, q_rows])
        m = np.zeros((16, 128, 256), np.float32)
        ar = np.arange(256)
        for s in range(4):
            j = 2 * s + role
            for rr in range(4):
                kt = 4 * s + rr
                m[s * 4 + rr] = ((128 * kt + np.arange(128)[:, None]) <=
                                 (256 * j + ar[None, :])).astype(np.float32)
        in_maps.append({
            "xT": xT.astype(bf),
            "xqT": xqT.astype(bf),
            "wq": wq_p,
            "wk": wk_p,
            "wv": wv_b,
            "wo": wo_b,
            "kcs": kcs.astype(bf),
            "qcs": qcs.astype(bf),
            "masks": m.astype(bf),
        })
    return in_maps


_NC_CACHE = {}


def kernel(x, wq, wk, wv, wo, freqs_cos, freqs_sin, mask_attention,
           start_pos=0, inference=0, **_ignored):
    from concourse.bass_utils import run_bass_kernel_spmd

    in_maps = make_in_maps(np.asarray(x, np.float32), wq, wk, wv, wo,
                           freqs_cos, freqs_sin)
    if "nc" not in _NC_CACHE:
        _NC_CACHE["nc"] = build_nc()
    nc = _NC_CACHE["nc"]
    res = run_bass_kernel_spmd(nc, in_maps, core_ids=list(range(8)))
    outs = res.results
    out_full = np.zeros((B, S, D), np.float32)
    for core in range(8):
        b, role = core // 2, core % 2
        zT = np.asarray(outs[core]["out"], np.float32)  # [768, 1024]
        for s in range(4):
            j = 2 * s + role
            out_full[b, 256 * j:256 * j + 256, :] = zT[:, 256 * s:256 * s + 256].T
    return out_full


# revision 26
# speedup vs baseline: 1.0005x; 1.0005x over previous
"""Trainium2 Bass kernel for GQA attention (B=4, S=2048, D=768, H=12, KVH=4, HD=64).

Sharding: 2 cores per batch. Each core computes all 12 heads for 4 query
chunks of 256 rows (role 0: chunks {0,2,4,6}, role 1: {1,3,5,7}) against the
full K/V of its batch, plus the complete wo projection for its rows. Output
shards are concatenated on the host; no collectives.

All cores run the same graph; causal structure differences between roles are
data-driven (host-built multiplicative masks applied to exp(scores)).

On-chip layout is transposed: qT/kT [head_dim, seq] (scores come out k-major),
V natural [seq, head_dim] with a 65th all-ones column per kv head so the PV
matmul also produces the softmax denominator in psum row 64 (no separate
denominator matmuls). Normalization: reciprocal on the psum denominator row,
DMA partition-broadcast of the reciprocal, elementwise multiply; odd heads of
each pair are shifted to partitions 64-127 of attnT by a small SBUF-to-SBUF
DMA. RoPE pairs are deinterleaved (host-permuted wq/wk columns) so the complex
rotation becomes out = cos*t + sign * SWAP @ (sin*t) on 1024-wide chunks.
"""

import sys

if "/opt/trn_rl_repo" not in sys.path:
    sys.path.insert(0, "/opt/trn_rl_repo")

import numpy as np
import ml_dtypes

import concourse.bass as bass
import concourse.tile as tile
from concourse import bacc, mybir

F32 = mybir.dt.float32
BF16 = mybir.dt.bfloat16

B, S, D = 4, 2048, 768
H, KVH, HD = 12, 4, 64

PAIRS = [(0, 1), (2, 3), (4, 5), (6, 7), (8, 9), (10, 11)]
GROUPS = [(0, 1), (2, 3), (4, 5)]  # pair indices per exp-batch group


def _kv(h):
    return h // 3


def _dram_view(base_ap, ap_list, extra_offset=0):
    return bass.AP(tensor=base_ap.tensor, offset=base_ap.offset + extra_offset,
                   ap=ap_list)


def build_nc(phases=3):
    nc = bacc.Bacc(None, target_bir_lowering=False)

    xT = nc.dram_tensor("xT", [D, S], BF16, kind="ExternalInput")
    xqT = nc.dram_tensor("xqT", [D, 1024], BF16, kind="ExternalInput")
    wq = nc.dram_tensor("wq", [D, H * HD], BF16, kind="ExternalInput")
    wk = nc.dram_tensor("wk", [D, KVH * HD], BF16, kind="ExternalInput")
    wv = nc.dram_tensor("wv", [D, KVH * HD], BF16, kind="ExternalInput")
    wo = nc.dram_tensor("wo", [H * HD, D], BF16, kind="ExternalInput")
    kcs = nc.dram_tensor("kcs", [64, S], BF16, kind="ExternalInput")  # [cos;sin]
    qcs = nc.dram_tensor("qcs", [64, 1024], BF16, kind="ExternalInput")
    masks = nc.dram_tensor("masks", [16, 128, 256], BF16, kind="ExternalInput")
    out = nc.dram_tensor("out", [D, 1024], BF16, kind="ExternalOutput")

    # swap matrix: exchanges 32-partition blocks 0<->1, 2<->3
    SW = np.zeros((128, 128), ml_dtypes.bfloat16)
    for blk in range(4):
        srcb = blk ^ 1
        for i in range(32):
            SW[blk * 32 + i, srcb * 32 + i] = 1.0
    sw_dram = nc.inline_tensor(SW, name="swconst")
    sign = np.zeros((128, 1), np.float32)
    for blk in range(4):
        sign[blk * 32:(blk + 1) * 32] = -1.0 if blk % 2 == 0 else 1.0
    sign_dram = nc.inline_tensor(sign, name="signconst")
    id_dram = nc.inline_tensor(np.eye(128, dtype=ml_dtypes.bfloat16),
                               name="idconst")

    def mm(out_ap, lhsT, rhs, start, stop, tile_position=None):
        nc.tensor.matmul(
            out_ap, lhsT, rhs,
            start=start, stop=stop,
            tile_position=tile_position,
            skip_group_check=True,
        )

    with tile.TileContext(nc) as tc:
        with tc.tile_pool(name="persist", bufs=1) as persist:
            qT64 = persist.tile([64, H, 1024], BF16)
            kT64 = persist.tile([64, KVH, S], BF16)
            # V with a 65th all-ones column per kv head (denominator trick)
            V2 = persist.tile([128, 16, KVH, 65], BF16)
            wk_sb = persist.tile([128, 6, 256], BF16)
            wv_sb = persist.tile([128, 6, 256], BF16)
            wo_sb = persist.tile([128, 6, D], BF16)
            sw_sb = persist.tile([128, 128], BF16)
            id_sb = persist.tile([128, 128], BF16)
            sign_sb = persist.tile([128, 1], F32)

            def load_rows(dst, src_dram, ncols, nblk, col0=0, width=None):
                # [nblk*128, ncols] dram -> [128, nblk, width] sbuf.
                # One DMA per 128-row block: spreads across HW queues.
                width = ncols if width is None else width
                for blk in range(nblk):
                    src = _dram_view(
                        src_dram[:, :], [[ncols, 128], [1, width]],
                        extra_offset=col0 + blk * 128 * ncols)
                    nc.sync.dma_start(out=dst[:, blk, :], in_=src)

            nc.vector.memset(V2[:, :, :, :], 1.0)  # ones col survives at 64
            load_rows(wk_sb[:, :, :], wk, 256, 6)
            load_rows(wv_sb[:, :, :], wv, 256, 6)
            nc.sync.dma_start(out=sw_sb[:, :], in_=sw_dram[:, :])
            nc.sync.dma_start(out=id_sb[:, :], in_=id_dram[:, :])
            nc.sync.dma_start(out=sign_sb[:, :], in_=sign_dram[:, :])

            # ---------------- Phase 1: projections + rope ----------------
            with tc.tile_pool(name="p1", bufs=1) as p1, \
                 tc.tile_pool(name="cs", bufs=2) as csp, \
                 tc.tile_pool(name="tmp", bufs=3) as tmpp, \
                 tc.tile_pool(name="rop", bufs=3) as ropp, \
                 tc.tile_pool(name="psA", bufs=2, space="PSUM") as psA, \
                 tc.tile_pool(name="psB", bufs=2, space="PSUM") as psB:

                xT_sb = p1.tile([128, 6, S], BF16)
                xqT_sb = p1.tile([128, 6, 1024], BF16)
                wq_sb = p1.tile([128, 6, H * HD], BF16)
                cs_q = p1.tile([128, 2, 1024], BF16)

                load_rows(xT_sb[:, :, :], xT, S, 6)
                load_rows(xqT_sb[:, :, :], xqT, 1024, 6)
                load_rows(wq_sb[:, :, :], wq, H * HD, 6)
                load_rows(wo_sb[:, :, :], wo, D, 6)

                def cs_load(dst, src_dram, col0, width):
                    # [cos(32 rows); sin(32 rows)] x width -> [128, 2, width]
                    # partitions replicate each 32-row block 4x
                    ncols = src_dram.shape[1]
                    for cs in range(2):
                        src = _dram_view(
                            src_dram[:, :],
                            [[0, 4], [ncols, 32], [1, width]],
                            extra_offset=col0 + cs * 32 * ncols)
                        nc.sync.dma_start(out=dst[:, cs, :], in_=src)

                warm_ps = psA.tile([128, 1024], F32, tag="pj")
                for wi in range(24):
                    mm(warm_ps[:, 0:128], id_sb[:, :], id_sb[:, :],
                       start=(wi == 0), stop=(wi == 23))
                warm_sb = tmpp.tile([128, 1024], F32, tag="ta")
                nc.scalar.activation(
                    out=warm_sb[:, 0:128], in_=warm_ps[:, 0:128],
                    func=mybir.ActivationFunctionType.Exp, scale=0.01)

                def rope_chunk(proj_ps, cs_t):
                    # returns bf16 [128, 1024]: cos*t + sign * SWAP @ (sin*t)
                    ta = tmpp.tile([128, 1024], F32, tag="ta")
                    tb = tmpp.tile([128, 1024], BF16, tag="tb")
                    nc.vector.tensor_mul(ta[:, :], proj_ps, cs_t[:, 0, :])
                    nc.vector.tensor_mul(tb[:, :], proj_ps, cs_t[:, 1, :])
                    sw_ps = psB.tile([128, 1024], F32, tag="sw")
                    mm(sw_ps[:, 0:512], sw_sb[:, :], tb[:, 0:512],
                       start=True, stop=True)
                    mm(sw_ps[:, 512:1024], sw_sb[:, :], tb[:, 512:1024],
                       start=True, stop=True)
                    ro = ropp.tile([128, 1024], BF16, tag="ro")
                    nc.vector.scalar_tensor_tensor(
                        out=ro[:, :],
                        in0=sw_ps[:, :],
                        scalar=sign_sb[:, 0:1],
                        in1=ta[:, :],
                        op0=mybir.AluOpType.mult,
                        op1=mybir.AluOpType.add,
                    )
                    return ro

                # K projection + rope: m-tile = kv pair (2m, 2m+1); 2 chunks
                for c in range(2):
                    cs_t = csp.tile([128, 2, 1024], BF16, tag="cs")
                    cs_load(cs_t[:, :, :], kcs, c * 1024, 1024)
                    for m in range(2):
                        k_ps = psA.tile([128, 1024], F32, tag="pj")
                        for dt in range(6):
                            for hf in range(2):
                                mm(k_ps[:, hf * 512:(hf + 1) * 512],
                                   wk_sb[:, dt, m * 128:(m + 1) * 128],
                                   xT_sb[:, dt, c * 1024 + hf * 512:
                                         c * 1024 + (hf + 1) * 512],
                                   start=(dt == 0), stop=(dt == 5))
                        ro = rope_chunk(k_ps[:, :], cs_t)
                        nc.sync.dma_start(
                            out=kT64[:, 2 * m, c * 1024:(c + 1) * 1024],
                            in_=ro[0:64, :])
                        nc.sync.dma_start(
                            out=kT64[:, 2 * m + 1, c * 1024:(c + 1) * 1024],
                            in_=ro[64:128, :])

                # Q projection + rope: m-tile = head pair (2t, 2t+1)
                cs_load(cs_q[:, :, :], qcs, 0, 1024)
                for t in range(6):
                    q_ps = psA.tile([128, 1024], F32, tag="pj")
                    for dt in range(6):
                        for hf in range(2):
                            mm(q_ps[:, hf * 512:(hf + 1) * 512],
                               wq_sb[:, dt, t * 128:(t + 1) * 128],
                               xqT_sb[:, dt, hf * 512:(hf + 1) * 512],
                               start=(dt == 0), stop=(dt == 5))
                    ro = rope_chunk(q_ps[:, :], cs_q)
                    nc.sync.dma_start(out=qT64[:, 2 * t, :], in_=ro[0:64, :])
                    nc.sync.dma_start(out=qT64[:, 2 * t + 1, :],
                                      in_=ro[64:128, :])

                # V projection (natural layout): 16 seq tiles, 4 per psum
                for v4 in range(4):
                    v_ps = psA.tile([128, 4, 4, 64], F32, tag="pj")
                    for st4 in range(4):
                        st = v4 * 4 + st4
                        for dt in range(6):
                            mm(v_ps[:, st4, :, :],
                               xT_sb[:, dt, st * 128:(st + 1) * 128],
                               wv_sb[:, dt, :], start=(dt == 0),
                               stop=(dt == 5))
                    if phases != 7:  # 7: debug, leave V2 = memset ones
                        for st4 in range(4):
                            nc.scalar.copy(V2[:, v4 * 4 + st4, :, 0:64],
                                           v_ps[:, st4, :, :])

            if phases == 1:
                for t in range(6):
                    nc.sync.dma_start(
                        out=out[t * 128:t * 128 + 64, :],
                        in_=qT64[:, 2 * t, :])
                    nc.sync.dma_start(
                        out=out[t * 128 + 64:(t + 1) * 128, :],
                        in_=qT64[:, 2 * t + 1, :])

            if phases in (6, 7):  # debug: dump V2 (first 6 seq tiles)
                for st in range(6):
                    nc.sync.dma_start(
                        out=out[st * 128:(st + 1) * 128, 0:260],
                        in_=V2[:, st, :, :])

            # ---------------- Phase 2: attention ----------------
            if 2 <= phases <= 5:
              with tc.tile_pool(name="p2", bufs=1) as p2, \
                 tc.tile_pool(name="expp", bufs=3) as expp, \
                 tc.tile_pool(name="nrm", bufs=3) as nrm, \
                 tc.tile_pool(name="zsb", bufs=2) as zsb, \
                 tc.tile_pool(name="recdp", bufs=3, space="DRAM") as recdp, \
                 tc.tile_pool(name="psSC", bufs=2, space="PSUM") as psSC, \
                 tc.tile_pool(name="psPV", bufs=4, space="PSUM") as psPV:

                masks_sb = p2.tile([128, 16, 256], BF16)
                attnT = p2.tile([128, 6, 1024], BF16)
                for mh in range(2):
                    src = _dram_view(masks[:, :, :],
                                     [[256, 128], [128 * 256, 8], [1, 256]],
                                     extra_offset=mh * 8 * 128 * 256)
                    nc.sync.dma_start(out=masks_sb[:, mh * 8:(mh + 1) * 8, :],
                                      in_=src)

                def rep4(base):
                    # [128, 256] -> [128, 4, 256] via stride-0 free dim
                    return bass.AP(tensor=base.tensor, offset=base.offset,
                                   ap=[list(base.ap)[0], [0, 4], [1, 256]])

                def quad(base):
                    # [128, 1024] contiguous -> [128, 4, 256] view
                    return bass.AP(tensor=base.tensor, offset=base.offset,
                                   ap=[list(base.ap)[0], [256, 4], [1, 256]])

                # deferred wo emission: one mi-tile per call
                def wo_mi(c, mi, z_all):
                    z_ps = psSC.tile([128, 256], F32, tag="sc")
                    for hd in range(6):
                        mm(z_ps[:, :],
                           wo_sb[:, hd, mi * 128:(mi + 1) * 128],
                           attnT[:, hd, c * 256:(c + 1) * 256],
                           start=(hd == 0), stop=(hd == 5))
                    nc.vector.tensor_copy(z_all[:, mi, :], z_ps[:, :])

                def wo_flush(c, z_all):
                    dst = _dram_view(out[:, :],
                                     [[1024, 128], [128 * 1024, 6], [1, 256]],
                                     extra_offset=c * 256)
                    nc.sync.dma_start(out=dst, in_=z_all[:, :, :])

                pending_wo = []  # ("mi"|"flush", fn) emission queue

                def pop_wo(n):
                    done = 0
                    while pending_wo and done < n:
                        kind, fn = pending_wo[0]
                        if kind == "mi":
                            done += 1
                        pending_wo.pop(0)
                        fn()

                for s in range(4):
                    n_kt = 4 * s + 4
                    for gi, grp in enumerate(GROUPS):
                        pvt = [psPV.tile([128, 512], F32, tag="pv",
                                         name=f"pv{s}{gi}{pi}")
                               for pi in range(2)]
                        for kt in range(n_kt):
                            if kt % 2 == 0:
                                expT2 = expp.tile([128, 2, 1024], BF16,
                                                  tag="expT")
                            sc_ps = psSC.tile([128, 1024], F32, tag="sc")
                            masked = kt >= 4 * s
                            # scores (transposed): [k 128, q 256] per head
                            for pi, p in enumerate(grp):
                                hA, hB = PAIRS[p]
                                o = pi * 512
                                if _kv(hA) == _kv(hB):
                                    mm(sc_ps[:, o:o + 512],
                                       kT64[:, _kv(hA),
                                            kt * 128:(kt + 1) * 128],
                                       qT64[:, hA:hA + 2,
                                            s * 256:(s + 1) * 256],
                                       start=True, stop=True)
                                else:
                                    for half, h in enumerate((hA, hB)):
                                        mm(sc_ps[:, o + half * 256:
                                                 o + half * 256 + 256],
                                           kT64[:, _kv(h),
                                                kt * 128:(kt + 1) * 128],
                                           qT64[:, h,
                                                s * 256:(s + 1) * 256],
                                           start=(half == 0), stop=True)
                            # exp (scale 1/8) from psum -> sbuf
                            expT = expT2[:, kt % 2, :]
                            nc.scalar.activation(
                                out=expT,
                                in_=sc_ps[:, :],
                                func=mybir.ActivationFunctionType.Exp,
                                scale=0.125,
                            )
                            if masked:
                                midx = s * 4 + (kt - 4 * s)
                                nc.vector.tensor_mul(
                                    quad(expT), quad(expT),
                                    rep4(masks_sb[:, midx, :]))
                            # PV with ones column: row 64 = denominator
                            for pi, p in enumerate(grp):
                                hA, hB = PAIRS[p]
                                kvA, kvB = _kv(hA), _kv(hB)
                                o = pi * 512
                                if kvA == kvB:
                                    mm(pvt[pi][0:65, 0:512],
                                       V2[:, kt, kvA, :],
                                       expT2[:, kt % 2, o:o + 512],
                                       start=(kt == 0),
                                       stop=(kt == n_kt - 1))
                                else:
                                    mm(pvt[pi][0:65, 0:256],
                                       V2[:, kt, kvA, :],
                                       expT2[:, kt % 2, o:o + 256],
                                       start=(kt == 0),
                                       stop=(kt == n_kt - 1))
                                    mm(pvt[pi][0:65, 256:512],
                                       V2[:, kt, kvB, :],
                                       expT2[:, kt % 2, o + 256:o + 512],
                                       start=False,
                                       stop=(kt == n_kt - 1))
                        # deferred wo matmuls first: they read attnT chunks
                        # from a PREVIOUS s, so they keep TensorE busy while
                        # this group's normalize chain runs
                        pop_wo(2)
                        # ---- normalize group ----
                        denb = nrm.tile([64, 2, 512], F32, tag="denb",
                                        name=f"denb{s}{gi}")
                        recb = nrm.tile([64, 2, 512], F32, tag="recb",
                                        name=f"recb{s}{gi}")
                        tmpB = nrm.tile([64, 2, 256], BF16, tag="tmpB",
                                        name=f"tmpB{s}{gi}")
                        # denominator rows (psum lane 64) -> DRAM bounce ->
                        # broadcast to 64 partitions, then reciprocal at base 0
                        # (reciprocal_approx_fast misbehaves off partition 0)
                        recd_t = recdp.tile([2, 512], F32, tag="recd",
                                            name=f"recd{s}{gi}")
                        den_sb = nrm.tile([128, 2, 512], F32, tag="den",
                                          name=f"den{s}{gi}")
                        for pi in range(2):
                            nc.vector.tensor_copy(den_sb[64:65, pi, :],
                                                  pvt[pi][64:65, 0:512])
                        nc.sync.dma_start(out=recd_t[:, :],
                                          in_=den_sb[64:65, :, :])
                        base = recd_t[:, :]
                        rep = bass.AP(tensor=base.tensor, offset=base.offset,
                                      ap=[[0, 64]] + list(base.ap))
                        nc.sync.dma_start(out=denb[:, :, :], in_=rep)
                        nc.vector.reciprocal_approx_fast(
                            recb[:, :, :], denb[:, :, :])
                        qs = slice(s * 256, (s + 1) * 256)
                        for pi, p in enumerate(grp):
                            nc.vector.tensor_mul(
                                attnT[0:64, p, qs],
                                pvt[pi][0:64, 0:256], recb[:, pi, 0:256])
                            nc.vector.tensor_mul(
                                tmpB[:, pi, :],
                                pvt[pi][0:64, 256:512], recb[:, pi, 256:512])
                        # shift odd heads to partitions 64-127 of attnT
                        nc.sync.dma_start(
                            out=attnT[64:128, grp[0]:grp[0] + 2, qs],
                            in_=tmpB[:, :, :])
                    # queue wo chunk for this s (emitted at group boundaries)
                    if phases >= 3:
                        z_all = zsb.tile([128, 6, 256], BF16, tag="z",
                                         name=f"zall{s}")
                        for mi in range(6):
                            pending_wo.append(
                                ("mi", lambda c=s, mi=mi, z=z_all:
                                 wo_mi(c, mi, z)))
                        pending_wo.append(
                            ("flush", lambda c=s, z=z_all: wo_flush(c, z)))
                # drain remaining wo work (last s)
                for _, fn in pending_wo:
                    fn()
                pending_wo = []

                if phases == 2:
                    for t in range(6):
                        nc.sync.dma_start(
                            out=out[t * 128:(t + 1) * 128, :],
                            in_=attnT[:, t, :])

    nc.compile()
    return nc


# ---------------------------------------------------------------------------
# host side
# ---------------------------------------------------------------------------

def _permute_cols(w, nheads):
    """Deinterleave rope pairs within each head: even dims then odd dims."""
    cols = []
    for h in range(nheads):
        blk = w[:, h * HD:(h + 1) * HD]
        cols.append(blk[:, 0::2])
        cols.append(blk[:, 1::2])
    return np.ascontiguousarray(np.concatenate(cols, axis=1))


def make_in_maps(x, wq, wk, wv, wo, freqs_cos, freqs_sin):
    bf = ml_dtypes.bfloat16
    wq_p = _permute_cols(np.asarray(wq, np.float32), H).astype(bf)
    wk_p = _permute_cols(np.asarray(wk, np.float32), KVH).astype(bf)
    wv_b = np.ascontiguousarray(np.asarray(wv, np.float32)).astype(bf)
    wo_b = np.ascontiguousarray(np.asarray(wo, np.float32)).astype(bf)

    cosT = np.ascontiguousarray(np.asarray(freqs_cos, np.float32).T)  # [32, S]
    sinT = np.ascontiguousarray(np.asarray(freqs_sin, np.float32).T)
    kcs = np.ascontiguousarray(np.concatenate([cosT, sinT], axis=0))  # [64, S]

    in_maps = []
    for core in range(8):
        b, role = core // 2, core % 2
        xT = np.ascontiguousarray(np.asarray(x[b], np.float32).T)
        q_rows = np.concatenate(
            [np.arange(256 * (2 * s + role), 256 * (2 * s + role) + 256)
             for s in range(4)])
        xqT = np.ascontiguousarray(xT[:, q_rows])
        qcs = np.ascontiguousarray(kcs[:, q_rows])
        m = np.zeros((16, 128, 256), np.float32)
        ar = np.arange(256)
        for s in range(4):
            j = 2 * s + role
            for rr in range(4):
                kt = 4 * s + rr
                m[s * 4 + rr] = ((128 * kt + np.arange(128)[:, None]) <=
                                 (256 * j + ar[None, :])).astype(np.float32)
        in_maps.append({
            "xT": xT.astype(bf),
            "xqT": xqT.astype(bf),
            "wq": wq_p,
            "wk": wk_p,
            "wv": wv_b,
            "wo": wo_b,
            "kcs": kcs.astype(bf),
            "qcs": qcs.astype(bf),
            "masks": m.astype(bf),
        })
    return in_maps


_NC_CACHE = {}


def kernel(x, wq, wk, wv, wo, freqs_cos, freqs_sin, mask_attention,
           start_pos=0, inference=0, **_ignored):
    from concourse.bass_utils import run_bass_kernel_spmd

    in_maps = make_in_maps(np.asarray(x, np.float32), wq, wk, wv, wo,
                           freqs_cos, freqs_sin)
    if "nc" not in _NC_CACHE:
        _NC_CACHE["nc"] = build_nc()
    nc = _NC_CACHE["nc"]
    res = run_bass_kernel_spmd(nc, in_maps, core_ids=list(range(8)))
    outs = res.results
    out_full = np.zeros((B, S, D), np.float32)
    for core in range(8):
        b, role = core // 2, core % 2
        zT = np.asarray(outs[core]["out"], np.float32)  # [768, 1024]
        for s in range(4):
            j = 2 * s + role
            out_full[b, 256 * j:256 * j + 256, :] = zT[:, 256 * s:256 * s + 256].T
    return out_full


# revision 33
# speedup vs baseline: 1.0535x; 1.0530x over previous
"""Trainium2 Bass kernel for GQA attention (B=4, S=2048, D=768, H=12, KVH=4, HD=64).

Sharding: 2 cores per batch. Each core computes all 12 heads for 4 query
chunks of 256 rows (role 0: chunks {0,2,4,6}, role 1: {1,3,5,7}) against the
full K/V of its batch, plus the complete wo projection for its rows. Output
shards are concatenated on the host; no collectives.

All cores run the same graph; causal structure differences between roles are
data-driven (host-built multiplicative masks applied to exp(scores)).

On-chip layout is transposed: qT/kT [head_dim, seq] (scores come out k-major),
V natural [seq, head_dim] with a 65th all-ones column per kv head so the PV
matmul also produces the softmax denominator in psum row 64 (no separate
denominator matmuls). Normalization: reciprocal on the psum denominator row,
DMA partition-broadcast of the reciprocal, elementwise multiply; odd heads of
each pair are shifted to partitions 64-127 of attnT by a small SBUF-to-SBUF
DMA. RoPE pairs are deinterleaved (host-permuted wq/wk columns) so the complex
rotation becomes out = cos*t + sign * SWAP @ (sin*t) on 1024-wide chunks.
"""

import sys

if "/opt/trn_rl_repo" not in sys.path:
    sys.path.insert(0, "/opt/trn_rl_repo")

import numpy as np
import ml_dtypes

import concourse.bass as bass
import concourse.tile as tile
from concourse import bacc, mybir

F32 = mybir.dt.float32
BF16 = mybir.dt.bfloat16

B, S, D = 4, 2048, 768
H, KVH, HD = 12, 4, 64

PAIRS = [(0, 1), (2, 3), (4, 5), (6, 7), (8, 9), (10, 11)]
GROUPS = [(0, 1), (2, 3), (4, 5)]  # pair indices per exp-batch group


def _kv(h):
    return h // 3


def _dram_view(base_ap, ap_list, extra_offset=0):
    return bass.AP(tensor=base_ap.tensor, offset=base_ap.offset + extra_offset,
                   ap=ap_list)


def build_nc(phases=3):
    nc = bacc.Bacc(None, target_bir_lowering=False)

    xT = nc.dram_tensor("xT", [D, S], BF16, kind="ExternalInput")
    xqT = nc.dram_tensor("xqT", [D, 1024], BF16, kind="ExternalInput")
    wq = nc.dram_tensor("wq", [D, H * HD], BF16, kind="ExternalInput")
    wk = nc.dram_tensor("wk", [D, KVH * HD], BF16, kind="ExternalInput")
    wv = nc.dram_tensor("wv", [D, KVH * HD], BF16, kind="ExternalInput")
    wo = nc.dram_tensor("wo", [H * HD, D], BF16, kind="ExternalInput")
    kcs = nc.dram_tensor("kcs", [64, S], BF16, kind="ExternalInput")  # [cos;sin]
    qcs = nc.dram_tensor("qcs", [64, 1024], BF16, kind="ExternalInput")
    masks = nc.dram_tensor("masks", [16, 128, 256], BF16, kind="ExternalInput")
    out = nc.dram_tensor("out", [D, 1024], BF16, kind="ExternalOutput")

    # swap matrix: exchanges 32-partition blocks 0<->1, 2<->3
    SW = np.zeros((128, 128), ml_dtypes.bfloat16)
    for blk in range(4):
        srcb = blk ^ 1
        for i in range(32):
            SW[blk * 32 + i, srcb * 32 + i] = 1.0
    sw_dram = nc.inline_tensor(SW, name="swconst")
    sign = np.zeros((128, 1), np.float32)
    for blk in range(4):
        sign[blk * 32:(blk + 1) * 32] = -1.0 if blk % 2 == 0 else 1.0
    sign_dram = nc.inline_tensor(sign, name="signconst")
    id_dram = nc.inline_tensor(np.eye(128, dtype=ml_dtypes.bfloat16),
                               name="idconst")

    def mm(out_ap, lhsT, rhs, start, stop, tile_position=None):
        nc.tensor.matmul(
            out_ap, lhsT, rhs,
            start=start, stop=stop,
            tile_position=tile_position,
            skip_group_check=True,
        )

    with tile.TileContext(nc) as tc:
        with tc.tile_pool(name="persist", bufs=1) as persist:
            qT64 = persist.tile([64, H, 1024], BF16)
            kT64 = persist.tile([64, KVH, S], BF16)
            # V with a 65th all-ones column per kv head (denominator trick)
            V2 = persist.tile([128, 16, KVH, 65], BF16)
            wk_sb = persist.tile([128, 6, 256], BF16)
            wv_sb = persist.tile([128, 6, 256], BF16)
            wo_sb = persist.tile([128, 6, D], BF16)
            sw_sb = persist.tile([128, 128], BF16)
            id_sb = persist.tile([128, 128], BF16)
            sign_sb = persist.tile([128, 1], F32)

            def load_rows(dst, src_dram, ncols, nblk, per_dma=None):
                # [nblk*128, ncols] dram -> [128, nblk, ncols] sbuf.
                # per_dma 128-row blocks per trigger: balances the ~650ns
                # serialized trigger cost against per-queue bandwidth.
                per_dma = nblk if per_dma is None else per_dma
                for b0 in range(0, nblk, per_dma):
                    nb = min(per_dma, nblk - b0)
                    src = _dram_view(
                        src_dram[:, :],
                        [[ncols, 128], [128 * ncols, nb], [1, ncols]],
                        extra_offset=b0 * 128 * ncols)
                    nc.sync.dma_start(out=dst[:, b0:b0 + nb, :], in_=src)

            nc.vector.memset(V2[:, :, :, :], 1.0)  # ones col survives at 64
            load_rows(wk_sb[:, :, :], wk, 256, 6)
            load_rows(wv_sb[:, :, :], wv, 256, 6)
            nc.sync.dma_start(out=sw_sb[:, :], in_=sw_dram[:, :])
            nc.sync.dma_start(out=id_sb[:, :], in_=id_dram[:, :])
            nc.sync.dma_start(out=sign_sb[:, :], in_=sign_dram[:, :])

            # ---------------- Phase 1: projections + rope ----------------
            with tc.tile_pool(name="p1", bufs=1) as p1, \
                 tc.tile_pool(name="cs", bufs=2) as csp, \
                 tc.tile_pool(name="tmp", bufs=3) as tmpp, \
                 tc.tile_pool(name="rop", bufs=3) as ropp, \
                 tc.tile_pool(name="psA", bufs=2, space="PSUM") as psA, \
                 tc.tile_pool(name="psB", bufs=2, space="PSUM") as psB:

                xT_sb = p1.tile([128, 6, S], BF16)
                xqT_sb = p1.tile([128, 6, 1024], BF16)
                wq_sb = p1.tile([128, 6, H * HD], BF16)
                cs_q = p1.tile([128, 2, 1024], BF16)

                load_rows(xT_sb[:, :, :], xT, S, 6, per_dma=2)
                load_rows(xqT_sb[:, :, :], xqT, 1024, 6, per_dma=3)
                load_rows(wq_sb[:, :, :], wq, H * HD, 6, per_dma=3)
                load_rows(wo_sb[:, :, :], wo, D, 6)

                def cs_load(dst, src_dram, col0, width):
                    # [cos(32 rows); sin(32 rows)] x width -> [128, 2, width]
                    # partitions replicate each 32-row block 4x
                    ncols = src_dram.shape[1]
                    for cs in range(2):
                        src = _dram_view(
                            src_dram[:, :],
                            [[0, 4], [ncols, 32], [1, width]],
                            extra_offset=col0 + cs * 32 * ncols)
                        nc.sync.dma_start(out=dst[:, cs, :], in_=src)

                warm_ps = psA.tile([128, 1024], F32, tag="pj")
                for wi in range(24):
                    mm(warm_ps[:, 0:128], id_sb[:, :], id_sb[:, :],
                       start=(wi == 0), stop=(wi == 23))
                warm_sb = tmpp.tile([128, 1024], F32, tag="ta")
                nc.scalar.activation(
                    out=warm_sb[:, 0:128], in_=warm_ps[:, 0:128],
                    func=mybir.ActivationFunctionType.Exp, scale=0.01)

                def rope_chunk(proj_ps, cs_t, dst_even, dst_odd):
                    # rope = cos*t + sign * SWAP @ (sin*t); the even head
                    # (lanes 0-63) is written straight into its base-0 home,
                    # the odd head (lanes 64-127) lands in a temp and is
                    # DMA-shifted to base 0
                    ta = tmpp.tile([128, 1024], F32, tag="ta")
                    tb = tmpp.tile([128, 1024], BF16, tag="tb")
                    nc.vector.tensor_mul(ta[:, :], proj_ps, cs_t[:, 0, :])
                    nc.vector.tensor_mul(tb[:, :], proj_ps, cs_t[:, 1, :])
                    sw_ps = psB.tile([128, 1024], F32, tag="sw")
                    mm(sw_ps[:, 0:512], sw_sb[:, :], tb[:, 0:512],
                       start=True, stop=True)
                    mm(sw_ps[:, 512:1024], sw_sb[:, :], tb[:, 512:1024],
                       start=True, stop=True)
                    ro = ropp.tile([128, 1024], BF16, tag="ro")
                    nc.vector.scalar_tensor_tensor(
                        out=dst_even,
                        in0=sw_ps[0:64, :],
                        scalar=sign_sb[0:64, 0:1],
                        in1=ta[0:64, :],
                        op0=mybir.AluOpType.mult,
                        op1=mybir.AluOpType.add,
                    )
                    nc.vector.scalar_tensor_tensor(
                        out=ro[64:128, :],
                        in0=sw_ps[64:128, :],
                        scalar=sign_sb[64:128, 0:1],
                        in1=ta[64:128, :],
                        op0=mybir.AluOpType.mult,
                        op1=mybir.AluOpType.add,
                    )
                    nc.sync.dma_start(out=dst_odd, in_=ro[64:128, :])

                # K projection + rope: m-tile = kv pair (2m, 2m+1); 2 chunks
                for c in range(2):
                    cs_t = csp.tile([128, 2, 1024], BF16, tag="cs")
                    cs_load(cs_t[:, :, :], kcs, c * 1024, 1024)
                    for m in range(2):
                        k_ps = psA.tile([128, 1024], F32, tag="pj")
                        for dt in range(6):
                            for hf in range(2):
                                mm(k_ps[:, hf * 512:(hf + 1) * 512],
                                   wk_sb[:, dt, m * 128:(m + 1) * 128],
                                   xT_sb[:, dt, c * 1024 + hf * 512:
                                         c * 1024 + (hf + 1) * 512],
                                   start=(dt == 0), stop=(dt == 5))
                        rope_chunk(
                            k_ps[:, :], cs_t,
                            kT64[:, 2 * m, c * 1024:(c + 1) * 1024],
                            kT64[:, 2 * m + 1, c * 1024:(c + 1) * 1024])

                # Q projection + rope: m-tile = head pair (2t, 2t+1)
                cs_load(cs_q[:, :, :], qcs, 0, 1024)
                for t in range(6):
                    q_ps = psA.tile([128, 1024], F32, tag="pj")
                    for dt in range(6):
                        for hf in range(2):
                            mm(q_ps[:, hf * 512:(hf + 1) * 512],
                               wq_sb[:, dt, t * 128:(t + 1) * 128],
                               xqT_sb[:, dt, hf * 512:(hf + 1) * 512],
                               start=(dt == 0), stop=(dt == 5))
                    rope_chunk(q_ps[:, :], cs_q,
                               qT64[:, 2 * t, :], qT64[:, 2 * t + 1, :])

                # V projection (natural layout): 16 seq tiles, 4 per psum
                for v4 in range(4):
                    v_ps = psA.tile([128, 4, 4, 64], F32, tag="pj")
                    for st4 in range(4):
                        st = v4 * 4 + st4
                        for dt in range(6):
                            mm(v_ps[:, st4, :, :],
                               xT_sb[:, dt, st * 128:(st + 1) * 128],
                               wv_sb[:, dt, :], start=(dt == 0),
                               stop=(dt == 5))
                    if phases != 7:  # 7: debug, leave V2 = memset ones
                        for st4 in range(4):
                            nc.scalar.copy(V2[:, v4 * 4 + st4, :, 0:64],
                                           v_ps[:, st4, :, :])

            if phases == 1:
                for t in range(6):
                    nc.sync.dma_start(
                        out=out[t * 128:t * 128 + 64, :],
                        in_=qT64[:, 2 * t, :])
                    nc.sync.dma_start(
                        out=out[t * 128 + 64:(t + 1) * 128, :],
                        in_=qT64[:, 2 * t + 1, :])

            if phases in (6, 7):  # debug: dump V2 (first 6 seq tiles)
                for st in range(6):
                    nc.sync.dma_start(
                        out=out[st * 128:(st + 1) * 128, 0:260],
                        in_=V2[:, st, :, :])

            # ---------------- Phase 2: attention ----------------
            if 2 <= phases <= 5:
              with tc.tile_pool(name="p2", bufs=1) as p2, \
                 tc.tile_pool(name="expp", bufs=3) as expp, \
                 tc.tile_pool(name="nrm", bufs=3) as nrm, \
                 tc.tile_pool(name="zsb", bufs=2) as zsb, \
                 tc.tile_pool(name="recdp", bufs=3, space="DRAM") as recdp, \
                 tc.tile_pool(name="psSC", bufs=2, space="PSUM") as psSC, \
                 tc.tile_pool(name="psPV", bufs=4, space="PSUM") as psPV:

                masks_sb = p2.tile([128, 16, 256], BF16)
                attnT = p2.tile([128, 6, 1024], BF16)
                src = _dram_view(masks[:, :, :],
                                 [[256, 128], [128 * 256, 16], [1, 256]])
                nc.sync.dma_start(out=masks_sb[:, :, :], in_=src)

                def rep4(base):
                    # [128, 256] -> [128, 4, 256] via stride-0 free dim
                    return bass.AP(tensor=base.tensor, offset=base.offset,
                                   ap=[list(base.ap)[0], [0, 4], [1, 256]])

                def quad(base):
                    # [128, 1024] contiguous -> [128, 4, 256] view
                    return bass.AP(tensor=base.tensor, offset=base.offset,
                                   ap=[list(base.ap)[0], [256, 4], [1, 256]])

                # deferred wo emission: one mi-tile per call. z tiles live in
                # the pv ring so the scores ring never stalls on wo work
                def wo_mi(c, mi, z_all):
                    z_ps = psPV.tile([128, 256], F32, tag="pv")
                    for hd in range(6):
                        mm(z_ps[:, :],
                           wo_sb[:, hd, mi * 128:(mi + 1) * 128],
                           attnT[:, hd, c * 256:(c + 1) * 256],
                           start=(hd == 0), stop=(hd == 5))
                    nc.vector.tensor_copy(z_all[:, mi, :], z_ps[:, :])

                def wo_flush(c, z_all):
                    dst = _dram_view(out[:, :],
                                     [[1024, 128], [128 * 1024, 6], [1, 256]],
                                     extra_offset=c * 256)
                    nc.sync.dma_start(out=dst, in_=z_all[:, :, :])

                pending_wo = []  # ("mi"|"flush", fn) emission queue

                def pop_wo(n):
                    done = 0
                    while pending_wo and done < n:
                        kind, fn = pending_wo[0]
                        if kind == "mi":
                            done += 1
                        pending_wo.pop(0)
                        fn()

                for s in range(4):
                    n_kt = 4 * s + 4
                    for gi, grp in enumerate(GROUPS):
                        pvt = [psPV.tile([128, 512], F32, tag="pv",
                                         name=f"pv{s}{gi}{pi}")
                               for pi in range(2)]
                        for kt in range(n_kt):
                            if kt % 2 == 0:
                                expT2 = expp.tile([128, 2, 1024], BF16,
                                                  tag="expT")
                            sc_ps = psSC.tile([128, 1024], F32, tag="sc")
                            masked = kt >= 4 * s
                            # scores (transposed): [k 128, q 256] per head
                            for pi, p in enumerate(grp):
                                hA, hB = PAIRS[p]
                                o = pi * 512
                                if _kv(hA) == _kv(hB):
                                    mm(sc_ps[:, o:o + 512],
                                       kT64[:, _kv(hA),
                                            kt * 128:(kt + 1) * 128],
                                       qT64[:, hA:hA + 2,
                                            s * 256:(s + 1) * 256],
                                       start=True, stop=True)
                                else:
                                    for half, h in enumerate((hA, hB)):
                                        mm(sc_ps[:, o + half * 256:
                                                 o + half * 256 + 256],
                                           kT64[:, _kv(h),
                                                kt * 128:(kt + 1) * 128],
                                           qT64[:, h,
                                                s * 256:(s + 1) * 256],
                                           start=(half == 0), stop=True)
                            # exp (scale 1/8) from psum -> sbuf
                            expT = expT2[:, kt % 2, :]
                            nc.scalar.activation(
                                out=expT,
                                in_=sc_ps[:, :],
                                func=mybir.ActivationFunctionType.Exp,
                                scale=0.125,
                            )
                            if masked:
                                midx = s * 4 + (kt - 4 * s)
                                nc.vector.tensor_mul(
                                    quad(expT), quad(expT),
                                    rep4(masks_sb[:, midx, :]))
                            # PV with ones column: row 64 = denominator
                            for pi, p in enumerate(grp):
                                hA, hB = PAIRS[p]
                                kvA, kvB = _kv(hA), _kv(hB)
                                o = pi * 512
                                if kvA == kvB:
                                    mm(pvt[pi][0:65, 0:512],
                                       V2[:, kt, kvA, :],
                                       expT2[:, kt % 2, o:o + 512],
                                       start=(kt == 0),
                                       stop=(kt == n_kt - 1))
                                else:
                                    mm(pvt[pi][0:65, 0:256],
                                       V2[:, kt, kvA, :],
                                       expT2[:, kt % 2, o:o + 256],
                                       start=(kt == 0),
                                       stop=(kt == n_kt - 1))
                                    mm(pvt[pi][0:65, 256:512],
                                       V2[:, kt, kvB, :],
                                       expT2[:, kt % 2, o + 256:o + 512],
                                       start=False,
                                       stop=(kt == n_kt - 1))
                        # deferred wo matmuls first: they read attnT chunks
                        # from a PREVIOUS s, so they keep TensorE busy while
                        # this group's normalize chain runs
                        pop_wo(2)
                        # ---- normalize group ----
                        denb = nrm.tile([64, 2, 512], F32, tag="denb",
                                        name=f"denb{s}{gi}")
                        recb = nrm.tile([64, 2, 512], F32, tag="recb",
                                        name=f"recb{s}{gi}")
                        tmpB = nrm.tile([64, 2, 256], BF16, tag="tmpB",
                                        name=f"tmpB{s}{gi}")
                        # denominator rows (psum lane 64) -> DRAM bounce ->
                        # broadcast to 64 partitions, then reciprocal at base 0
                        # (reciprocal_approx_fast misbehaves off partition 0)
                        recd_t = recdp.tile([2, 512], F32, tag="recd",
                                            name=f"recd{s}{gi}")
                        den_sb = nrm.tile([128, 2, 512], F32, tag="den",
                                          name=f"den{s}{gi}")
                        for pi in range(2):
                            nc.vector.tensor_copy(den_sb[64:65, pi, :],
                                                  pvt[pi][64:65, 0:512])
                        nc.sync.dma_start(out=recd_t[:, :],
                                          in_=den_sb[64:65, :, :])
                        base = recd_t[:, :]
                        rep = bass.AP(tensor=base.tensor, offset=base.offset,
                                      ap=[[0, 64]] + list(base.ap))
                        nc.sync.dma_start(out=denb[:, :, :], in_=rep)
                        nc.vector.reciprocal_approx_fast(
                            recb[:, :, :], denb[:, :, :])
                        qs = slice(s * 256, (s + 1) * 256)
                        for pi, p in enumerate(grp):
                            nc.vector.tensor_mul(
                                attnT[0:64, p, qs],
                                pvt[pi][0:64, 0:256], recb[:, pi, 0:256])
                            nc.vector.tensor_mul(
                                tmpB[:, pi, :],
                                pvt[pi][0:64, 256:512], recb[:, pi, 256:512])
                        # shift odd heads to partitions 64-127 of attnT
                        nc.sync.dma_start(
                            out=attnT[64:128, grp[0]:grp[0] + 2, qs],
                            in_=tmpB[:, :, :])
                    # queue wo chunk for this s (emitted at group boundaries)
                    if phases >= 3:
                        z_all = zsb.tile([128, 6, 256], BF16, tag="z",
                                         name=f"zall{s}")
                        for mi in range(6):
                            pending_wo.append(
                                ("mi", lambda c=s, mi=mi, z=z_all:
                                 wo_mi(c, mi, z)))
                        pending_wo.append(
                            ("flush", lambda c=s, z=z_all: wo_flush(c, z)))
                # drain remaining wo work (last s)
                for _, fn in pending_wo:
                    fn()
                pending_wo = []

                if phases == 2:
                    for t in range(6):
                        nc.sync.dma_start(
                            out=out[t * 128:(t + 1) * 128, :],
                            in_=attnT[:, t, :])

    nc.compile()
    return nc


# ---------------------------------------------------------------------------
# host side
# ---------------------------------------------------------------------------

def _permute_cols(w, nheads):
    """Deinterleave rope pairs within each head: even dims then odd dims."""
    cols = []
    for h in range(nheads):
        blk = w[:, h * HD:(h + 1) * HD]
        cols.append(blk[:, 0::2])
        cols.append(blk[:, 1::2])
    return np.ascontiguousarray(np.concatenate(cols, axis=1))


def make_in_maps(x, wq, wk, wv, wo, freqs_cos, freqs_sin):
    bf = ml_dtypes.bfloat16
    wq_p = _permute_cols(np.asarray(wq, np.float32), H).astype(bf)
    wk_p = _permute_cols(np.asarray(wk, np.float32), KVH).astype(bf)
    wv_b = np.ascontiguousarray(np.asarray(wv, np.float32)).astype(bf)
    wo_b = np.ascontiguousarray(np.asarray(wo, np.float32)).astype(bf)

    cosT = np.ascontiguousarray(np.asarray(freqs_cos, np.float32).T)  # [32, S]
    sinT = np.ascontiguousarray(np.asarray(freqs_sin, np.float32).T)
    kcs = np.ascontiguousarray(np.concatenate([cosT, sinT], axis=0))  # [64, S]

    in_maps = []
    for core in range(8):
        b, role = core // 2, core % 2
        xT = np.ascontiguousarray(np.asarray(x[b], np.float32).T)
        q_rows = np.concatenate(
            [np.arange(256 * (2 * s + role), 256 * (2 * s + role) + 256)
             for s in range(4)])
        xqT = np.ascontiguousarray(xT[:, q_rows])
        qcs = np.ascontiguousarray(kcs[:, q_rows])
        m = np.zeros((16, 128, 256), np.float32)
        ar = np.arange(256)
        for s in range(4):
            j = 2 * s + role
            for rr in range(4):
                kt = 4 * s + rr
                m[s * 4 + rr] = ((128 * kt + np.arange(128)[:, None]) <=
                                 (256 * j + ar[None, :])).astype(np.float32)
        in_maps.append({
            "xT": xT.astype(bf),
            "xqT": xqT.astype(bf),
            "wq": wq_p,
            "wk": wk_p,
            "wv": wv_b,
            "wo": wo_b,
            "kcs": kcs.astype(bf),
            "qcs": qcs.astype(bf),
            "masks": m.astype(bf),
        })
    return in_maps


_NC_CACHE = {}


def kernel(x, wq, wk, wv, wo, freqs_cos, freqs_sin, mask_attention,
           start_pos=0, inference=0, **_ignored):
    from concourse.bass_utils import run_bass_kernel_spmd

    in_maps = make_in_maps(np.asarray(x, np.float32), wq, wk, wv, wo,
                           freqs_cos, freqs_sin)
    if "nc" not in _NC_CACHE:
        _NC_CACHE["nc"] = build_nc()
    nc = _NC_CACHE["nc"]
    res = run_bass_kernel_spmd(nc, in_maps, core_ids=list(range(8)))
    outs = res.results
    out_full = np.zeros((B, S, D), np.float32)
    for core in range(8):
        b, role = core // 2, core % 2
        zT = np.asarray(outs[core]["out"], np.float32)  # [768, 1024]
        for s in range(4):
            j = 2 * s + role
            out_full[b, 256 * j:256 * j + 256, :] = zT[:, 256 * s:256 * s + 256].T
    return out_full
